# revision 19
# baseline (speedup 1.0000x reference)
"""TRN2 Bass kernel for nn_AttentionMP (GNN message passing attention).

Row-parallel attention across 8 NeuronCores: core c owns query rows
[c*1024, (c+1)*1024). Scores are computed TRANSPOSED, sT[j, i] (j = key
index on partitions, i = this core's query rows on the free dim), so
att^T feeds downstream matmuls as the moving operand with no on-device
transposes. Data-independent products fold on the host: qk =
(Wq Wk^T)^T Hq^T ships per-core; the v-projection + first MLP layer
fold into the Z weights (below).

Denominator-in-Z: W1v = Wv@W1 is SVD-truncated to rank 127 (relative
tail 2.8e-5, negligible) so the Z accumulator's 128 PSUM rows hold
[den; A^T H^T e] where A = U[:,:127]*S[:127] and row 0 comes from a
ones-column spliced into the pretiled H-chunks. This removes the
serial per-tile DVE accumulation chain for the softmax denominator
entirely (~43us of DVE), accumulates den in f32 instead of bf16, and
folds b1 in for free (gps lhsT row 0 = b1, rows 1.. = B = Vt[:127]).

Masking (uniform exp bias -30 for every tile):
 - K1 tiles (front 5 + spread): additive on the PE — madj = 240*(adj-1)
   in {-240, 0} ships as fp8 and accumulates into the scores PSUM via an
   identity-stationary matmul; exp(s-240-30) underflows to 0 in bf16.
   Both mask halves issue BEFORE both score halves: an accumulating
   matmul (start=False) stalls ~420ns until its same-bank predecessor
   drains, and the [m0,m1,s0,s1] order hides that under other matmuls.
 - remaining tiles: multiplicative on the DVE from BITPACKED adjacency:
   adj ships as uint16 (bit b of group g = tile 16g+b -> 1MB instead of
   13MB), a both-bitwise tensor_scalar extracts (P>>b)&1 to uint16
   (~420ns), and a mixed uint16*bf16 tensor_mul masks e in place
   (~690ns, still 2x_1p).

exp runs at [128,1536] granularity from a 2-deep PSUM pool (3 banks per
buffer + 2 for Z = 8) into a 12-tile bf16 e-ring; the 222-cycle ACT
overhead amortizes over 1.5 tiles. ~14 dependency-free warmup matmuls
on memset zeros run into the Z banks during the DMA-gated start so the
PE clock gate (K=4/8 until ~3.4us of sustained activity) releases
before real work arrives; the first real Z matmul's start=True clears
the garbage. A dummy exp pulls ACT_TABLE_LOAD off the critical path.

Stage 2 (normalization deferred through the MLP since relu commutes
with positive row scaling): zsb <- Z PSUM, gps = [b1;B]^T zsb gives the
hidden pre-activations scaled by den, relu -> hts (bf16), and the final
layer re-flips per 128-row block (lhsT = hts block x W2) so relu with
the per-row 1/den scale (ACT/DVE alternating) writes the output staging
tile directly. 1/den columns come from tiny PE transposes of Z row 0.
"""
import numpy as np
import ml_dtypes
import concourse.bass as bass
from concourse import bacc
import concourse.mybir as mybir
from concourse.tile import TileContext
from concourse.bass_utils import run_bass_kernel_spmd

N = 8192
D = 128
NC = 8
RPC = N // NC          # rows per core = 1024
JT = N // 128          # j tiles = 64
F32 = mybir.dt.float32
F32R = mybir.dt.float32r
BF16 = mybir.dt.bfloat16
FP8 = mybir.dt.float8e4
U16 = mybir.dt.uint16
MASK_D = 240.0         # fp8e4 max finite
STAB = 30.0            # global score shift, cancels in softmax
HT_CHUNKS = 4
N_WARM = 8             # HAM-warmup matmuls at kernel start
CHUNK = 1536           # exp granularity (cols); 3 PSUM banks
NCHUNK = (JT * RPC + CHUNK - 1) // CHUNK   # 43 (last one 1024 wide)
RING = 12 * RPC        # e-ring: 12 tiles (LCM of 1024/1536 grain)
ZLAG = 3               # Z matmuls trail the score/exp front

K1 = [6, 12, 18, 24, 30, 36, 42, 48, 54, 60]   # PE-masked tiles (spread;
# the front tiles are DVE-masked so the first exp chunks depend only on
# qk/htc DMAs, not on madj/idf8)
K1_IDX = {jt: i for i, jt in enumerate(K1)}
NK1 = len(K1)

_CACHED = {}


def build(with_bias=False):
    nc = bacc.Bacc("TRN2", target_bir_lowering=False, debug=True)

    HTC = [nc.dram_tensor(f"HT{t}", [D, N // HT_CHUNKS], F32R, kind="ExternalInput")
           for t in range(HT_CHUNKS)]
    HNC = [nc.dram_tensor(f"HN{t}", [D, N // 4], BF16, kind="ExternalInput")
           for t in range(4)]  # pretiled [p, t*128+c], bf16; col c=0 is ones
    QK = nc.dram_tensor("QK", [D, RPC], F32R, kind="ExternalInput")
    MADJ = nc.dram_tensor("MADJ", [NK1 * 128, RPC], FP8, kind="ExternalInput")
    ADJP = nc.dram_tensor("ADJP", [D, 4 * RPC], U16, kind="ExternalInput")
    W1B = nc.dram_tensor("W1B", [D, D], BF16, kind="ExternalInput")
    W2 = nc.dram_tensor("W2", [D, D], BF16, kind="ExternalInput")
    B2R = nc.dram_tensor("B2R", [1, D], BF16, kind="ExternalInput")
    IDF8 = nc.dram_tensor("IDF8", [D, D], FP8, kind="ExternalInput")
    IDENT = nc.dram_tensor("IDENT", [1, 1], BF16, kind="ExternalInput")
    OUT = nc.dram_tensor("OUT", [RPC, D], F32, kind="ExternalOutput")

    madj_rows = MADJ.rearrange("(t p) i -> p t i", p=128)  # [p, K1-idx, i]

    with TileContext(nc) as tc:
        with (
            tc.tile_pool(name="pers", bufs=1) as pers,
            tc.tile_pool(name="madj", bufs=3) as madjp,
            tc.tile_pool(name="msk", bufs=4) as mskp,
            tc.tile_pool(name="psA", bufs=2, space="PSUM") as psA,   # 2x[128,1536]
            tc.tile_pool(name="psZ", bufs=1, space="PSUM") as psZ,   # Z accumulator
        ):
            # ---- persistent tiles
            htc = [pers.tile([D, N // HT_CHUNKS], F32R, tag=f"ht{t}",
                             name=f"htc{t}") for t in range(HT_CHUNKS)]
            hnc = [pers.tile([D, N // 4], BF16, tag=f"hn{t}", name=f"hnc{t}")
                   for t in range(4)]
            qk = pers.tile([D, RPC], F32R, tag="qk")
            adjp = pers.tile([D, 4 * RPC], U16, tag="adjp")
            w1b = pers.tile([D, D], BF16, tag="w1b")
            w2 = pers.tile([D, D], BF16, tag="w2")
            b2r = pers.tile([1, D], BF16, tag="b2r")
            idf8 = pers.tile([D, D], FP8, tag="idf8")
            ident = pers.tile([1, 1], BF16, tag="ident")
            ering = pers.tile([D, RING], BF16, tag="ering")
            zsb = pers.tile([D, RPC], BF16, tag="zsb")
            biasm = pers.tile([D, 1], F32, tag="biasm")
            hts = pers.tile([D, RPC], BF16, tag="hts")
            rcol = pers.tile([D, NC], F32, tag="rcol")
            outsb = pers.tile([D, NC * D], F32, tag="outsb")

            madj_sb = {}   # K1 tile -> (tile, ) fp8 [128, RPC]

            def madj_dma(jt, engine=None):
                t = madjp.tile([128, RPC], FP8, tag="ma", name=f"ma{jt}")
                madj_sb[jt] = t
                (engine or nc.sync).dma_start(out=t[:], in_=madj_rows[:, K1_IDX[jt]])

            # ---- critical-path DMAs, most-urgent first. The first exp
            # chunk needs only qk + htc0's head; packed adjacency and the
            # identity ride the gpsimd software queue in parallel.
            nc.sync.dma_start(out=qk[:, 0:512], in_=QK[:, 0:512])
            nc.sync.dma_start(out=htc[0][:, 0:256], in_=HTC[0][:, 0:256])
            nc.sync.dma_start(out=qk[:, 512:1024], in_=QK[:, 512:1024])
            nc.sync.dma_start(out=htc[0][:, 256:768], in_=HTC[0][:, 256:768])
            nc.sync.dma_start(out=hnc[0][:, 0:512], in_=HNC[0][:, 0:512])
            nc.gpsimd.dma_start(out=adjp[:, 0:2048], in_=ADJP[:, 0:2048])
            nc.gpsimd.dma_start(out=idf8[:], in_=IDF8[:])
            nc.sync.dma_start(out=htc[0][:, 768:N // HT_CHUNKS],
                              in_=HTC[0][:, 768:N // HT_CHUNKS])
            madj_dma(6)
            nc.gpsimd.dma_start(out=adjp[:, 2048:4096], in_=ADJP[:, 2048:4096])
            hchalf = N // HT_CHUNKS // 2
            nc.sync.dma_start(out=htc[1][:, 0:hchalf], in_=HTC[1][:, 0:hchalf])
            nc.sync.dma_start(out=hnc[0][:, 512:], in_=HNC[0][:, 512:])
            madj_dma(12)
            nc.sync.dma_start(out=htc[1][:, hchalf:], in_=HTC[1][:, hchalf:])
            for t, src in [(ident, IDENT), (w1b, W1B), (w2, W2), (b2r, B2R)]:
                nc.gpsimd.dma_start(out=t[:], in_=src[:])
            nc.gpsimd.dma_start(out=hnc[1][:], in_=HNC[1][:])

            # ---- Z accumulator + HAM warmup
            zps = psZ.tile([D, RPC], F32, tag="z")
            warm = pers.tile([D, 512], BF16, tag="warm")
            nc.vector.memset(warm[:], 0.0)
            nc.vector.memset(biasm[:], -STAB)
            nc.scalar.activation(warm[:, 0:1], warm[:, 0:1],
                                 mybir.ActivationFunctionType.Exp)
            for w in range(N_WARM):
                nc.tensor.matmul(zps[:, 0:512], lhsT=warm[:, 0:128],
                                 rhs=warm[:], start=True, stop=True)

            cwq = N // HT_CHUNKS // 128

            def do_z(t):
                htile = hnc[t // 16][:, (t % 16) * 128:(t % 16 + 1) * 128]
                base = (t % 12) * RPC
                for h in range(2):
                    nc.tensor.matmul(zps[:, h * 512:(h + 1) * 512], lhsT=htile,
                                     rhs=ering[:, base + h * 512:base + (h + 1) * 512],
                                     start=(t == 0), stop=(t == JT - 1))

            def dma_cadence(jt):
                # spread madj prefetch ~16 tiles ahead
                if (jt + 16) in K1_IDX:
                    madj_dma(jt + 16)
                for gc in (2, 3):
                    if jt == 16 * gc - 14:
                        nc.sync.dma_start(out=htc[gc][:, 0:hchalf],
                                          in_=HTC[gc][:, 0:hchalf])
                    elif jt == 16 * gc - 10:
                        nc.sync.dma_start(out=htc[gc][:, hchalf:],
                                          in_=HTC[gc][:, hchalf:])
                    elif jt == 16 * gc - 6:
                        nc.sync.dma_start(out=hnc[gc][:], in_=HNC[gc][:])

            z_next = 0
            t_proc = 0   # next tile awaiting mask processing
            for c in range(NCHUNK):
                start = c * CHUNK
                end = min(start + CHUNK, JT * RPC)
                width = end - start
                slices = [(col // RPC, (col % RPC) // 512)
                          for col in range(start, end, 512)]
                for jt, h in slices:
                    if h == 0:
                        dma_cadence(jt)
                sps = psA.tile([D, CHUNK], F32, tag="big")

                def score_mm(jt, h, masked):
                    col = jt * RPC + h * 512 - start
                    ktile = htc[jt // cwq][:, (jt % cwq) * 128:(jt % cwq + 1) * 128]
                    nc.tensor.matmul(sps[:, col:col + 512], lhsT=ktile,
                                     rhs=qk[:, h * 512:(h + 1) * 512],
                                     start=not masked, stop=True)

                # mask matmuls first, then unmasked scores, masked scores last
                for jt, h in slices:
                    if jt in K1_IDX:
                        col = jt * RPC + h * 512 - start
                        nc.tensor.matmul(sps[:, col:col + 512], lhsT=idf8[:],
                                         rhs=madj_sb[jt][:, h * 512:(h + 1) * 512],
                                         start=True, stop=False)
                for jt, h in slices:
                    if jt not in K1_IDX:
                        score_mm(jt, h, False)
                for jt, h in slices:
                    if jt in K1_IDX:
                        score_mm(jt, h, True)

                rp = start % RING
                nc.scalar.activation(ering[:, rp:rp + width], sps[:, 0:width],
                                     mybir.ActivationFunctionType.Exp,
                                     bias=biasm[:])

                # multiplicative masks for tiles fully covered by exps so far
                while (t_proc + 1) * RPC <= end:
                    t = t_proc
                    t_proc += 1
                    if t not in K1_IDX:
                        g, b = t // 16, t % 16
                        m = mskp.tile([D, RPC], U16, tag="m")
                        nc.vector.tensor_scalar(
                            m[:], adjp[:, g * RPC:(g + 1) * RPC],
                            float(b), 1.0,
                            op0=mybir.AluOpType.logical_shift_right,
                            op1=mybir.AluOpType.bitwise_and)
                        base = (t % 12) * RPC
                        nc.vector.tensor_mul(ering[:, base:base + RPC],
                                             ering[:, base:base + RPC], m[:])
                # Z trails the front by ZLAG tiles
                while z_next <= t_proc - 1 - ZLAG:
                    do_z(z_next)
                    z_next += 1
            while z_next < JT:
                do_z(z_next)
                z_next += 1

            # ---- stage 2: normalization-deferred transposed MLP.
            # Z row 0 is the softmax denominator (ones column of HN).
            nc.scalar.copy(zsb[:, 0:512], zps[:, 0:512])
            nc.vector.tensor_copy(zsb[:, 512:1024], zps[:, 512:1024])
            gps = psA.tile([D, CHUNK], F32, tag="big")
            for h in range(2):
                cs = slice(h * 512, (h + 1) * 512)
                nc.tensor.matmul(gps[:, cs], lhsT=w1b[:], rhs=zsb[:, cs],
                                 start=True, stop=True)
            # 1/denom columns via tiny PE transposes of the den row
            rps = psA.tile([D, CHUNK], BF16, tag="big")
            for it in range(NC):
                nc.tensor.transpose(rps[:, 2 * it:2 * it + 1],
                                    zsb[0:1, it * 128:(it + 1) * 128],
                                    ident[0:1, 0:1])
            nc.scalar.activation(hts[:, 0:512], gps[:, 0:512],
                                 mybir.ActivationFunctionType.Relu)
            nc.vector.tensor_relu(hts[:, 512:1024], gps[:, 512:1024])
            nc.vector.reciprocal(rcol[:], rps[:, 0:2 * NC:2])
            # final layer, re-flipped per 128-row block: lhsT = hts block so
            # the output block lands non-transposed in PSUM; relu + (1/d)
            # scale alternates ACT/DVE and writes the staging tile directly.
            outv = OUT.rearrange("(t p) d -> p t d", p=128)
            bpt = [psA.tile([D, CHUNK], F32, tag="big", name="bp0"),
                   psA.tile([D, CHUNK], F32, tag="big", name="bp1")]
            for it in range(NC):
                bps = bpt[it // 4][:, (it % 4) * D:(it % 4 + 1) * D]
                nc.tensor.matmul(bps, lhsT=hts[:, it * 128:(it + 1) * 128],
                                 rhs=w2[:], start=True, stop=not with_bias)
                if with_bias:
                    nc.tensor.matmul(bps,
                                     lhsT=zsb[0:1, it * 128:(it + 1) * 128],
                                     rhs=b2r[:], start=False, stop=True)
            for it in range(NC):
                bps = bpt[it // 4][:, (it % 4) * D:(it % 4 + 1) * D]
                ob = outsb[:, it * 128:(it + 1) * 128]
                if it % 2 == 0:
                    nc.scalar.activation(ob, bps,
                                         mybir.ActivationFunctionType.Relu,
                                         scale=rcol[:, it:it + 1])
                else:
                    nc.vector.tensor_scalar(ob, bps, rcol[:, it:it + 1],
                                            0.0, op0=mybir.AluOpType.mult,
                                            op1=mybir.AluOpType.max)
                if it % 2 == 1:
                    nc.sync.dma_start(
                        out=outv[:, it - 1:it + 1],
                        in_=outsb[:, (it - 1) * D:(it + 1) * D].rearrange(
                            "p (t d) -> p t d", t=2))
    nc.finalize()
    return nc


def _prep(H, adj, Wq, Wk, Wv, W1, b1, W2, b2):
    f8 = ml_dtypes.float8_e4m3
    bf = ml_dtypes.bfloat16
    H32 = np.asarray(H, dtype=np.float32)
    HT = np.ascontiguousarray(H32.T)
    adj = np.asarray(adj)
    M = (np.asarray(Wq, np.float32) @ np.asarray(Wk, np.float32).T)
    # SVD-truncate W1v = Wv@W1 to rank 127; den rides Z row 0 (ones col).
    W1v = np.asarray(Wv, np.float32) @ np.asarray(W1, np.float32)
    U, S, Vt = np.linalg.svd(W1v.astype(np.float64))
    A = (U[:, :127] * S[:127]).astype(np.float32)      # [128,127]
    B = Vt[:127].astype(np.float32)                    # [127,128]
    w1b = np.vstack([np.asarray(b1, np.float32).reshape(1, D), B])
    HA = np.concatenate([np.ones((N, 1), np.float32), H32 @ A], axis=1)
    base = {
        "W1B": np.ascontiguousarray(w1b).astype(bf),
        "W2": np.asarray(W2, np.float32).astype(bf),
        "B2R": np.asarray(b2, np.float32).reshape(1, D).astype(bf),
        "IDF8": np.eye(D, dtype=np.float32).astype(f8),
        "IDENT": np.eye(1, dtype=np.float32).astype(bf),
    }
    cw = N // HT_CHUNKS
    for t in range(HT_CHUNKS):
        base[f"HT{t}"] = np.ascontiguousarray(HT[:, t * cw:(t + 1) * cw])
    HNP = np.ascontiguousarray(
        HA.reshape(JT, 128, D).transpose(1, 0, 2).reshape(128, N)).astype(bf)
    for t in range(4):
        base[f"HN{t}"] = np.ascontiguousarray(HNP[:, t * (N // 4):(t + 1) * (N // 4)])
    in_maps = []
    for c in range(NC):
        m = dict(base)
        m["QK"] = np.ascontiguousarray(M.T @ HT[:, c * RPC:(c + 1) * RPC])
        adjT4 = np.ascontiguousarray(
            adj[c * RPC:(c + 1) * RPC, :].T).reshape(JT, 128, RPC)
        m["MADJ"] = np.ascontiguousarray(
            (adjT4[K1].astype(np.float32) - 1.0) * MASK_D
        ).reshape(NK1 * 128, RPC).astype(f8)
        packed = np.zeros((4, 128, RPC), np.uint16)
        for g in range(4):
            for b in range(16):
                packed[g] |= (adjT4[g * 16 + b] > 0).astype(np.uint16) << b
        m["ADJP"] = np.ascontiguousarray(
            packed.transpose(1, 0, 2).reshape(128, 4 * RPC))
        in_maps.append(m)
    return in_maps


def kernel(H, adj, Wq, Wk, Wv, W1, b1, W2, b2):
    wb = bool(np.any(np.asarray(b2)))
    key = f"nc{int(wb)}"
    if key not in _CACHED:
        _CACHED[key] = build(with_bias=wb)
    in_maps = _prep(H, adj, Wq, Wk, Wv, W1, b1, W2, b2)
    res = run_bass_kernel_spmd(_CACHED[key], in_maps, list(range(NC)))
    return np.concatenate([res.results[c]["OUT"] for c in range(NC)], axis=0)
